# revision 3
# baseline (speedup 1.0000x reference)
"""Trainium2 Bass kernel for the 1-D Bessel (von Mises-like) kernel matrix:

    K[i, j] = I0(2a * cos(pi * (x_i - y_j))) * exp(-2a),   a = 10

Algorithm
---------
K depends on d = x_i - y_j only through the periodic even function
h(d) = I0(20 cos(pi d)) e^-20, which has period 1.  Its log has a rapidly
converging Fourier cosine series:

    log h(d) = b0 + sum_{k=1..63} b_k cos(2 pi k d)          (|err| < 3e-8)

and cos(2 pi k (x - y)) = cos(2pi k x) cos(2pi k y) + sin(2pi k x) sin(2pi k y),
so log K is a rank-127 product of small trig feature matrices:

    log K = U.T @ V,   U, V in R^[128 x n]  (row 127 zero-padded)

On each NeuronCore (rows of x sharded 8 ways, y replicated): the rank-128
contraction runs on the TensorEngine as a 3-pass split-bf16 matmul
(U = Uh + Ul, V = Vh + Vl exactly in bf16 pairs; U.T V ~ Uh.T Vh + Uh.T Vl
+ Ul.T Vh accumulated in fp32 PSUM, ~1e-4 max rel err), then a fused
exp() on the Scalar engine moves PSUM->SBUF, and 4 MiB DMAs write each
128-row block to HBM.  The run is output-DMA-bound (~32 MiB/core at
~358 GB/s, ~94 us); the 3 bf16 matmul passes (~92 us) hide under the DMA.

The tiny [128 x 8192] trig features are precomputed on host in float64.
"""

import os
import sys

import numpy as np

sys.path.insert(0, "/opt/trn_rl_repo")

A = 10.0
NX = 8192
NY = 8192
N_CORES = 8
MX = NX // N_CORES  # 1024 rows of x per core
KH = 63  # harmonics; rank = 1 + 2*63 = 127 (+1 zero pad = 128)

# Fourier cosine coefficients of log(I0(20 cos(pi d))) - 20 on d in [0, 1),
# computed offline in float64 via FFT of the exact series evaluation.
_B0 = -9.320623105523872
_BK = [
    7.970447139028089, -1.4358756600553582, 0.5530401566383198,
    -0.27432647869384885, 0.1547723650507224, -0.09433791302730635,
    0.060502068515108406, -0.04020530135648252, 0.027418113277826187,
    -0.01906554834357182, 0.013458315954332174, -0.009613552975863679,
    0.0069329638057468446, -0.005038947804517573, 0.003686131354141929,
    -0.00271122806102214, 0.00200343687917714, -0.0014863506699641636,
    0.00110656955440988, -0.0008263523699001975, 0.000618771677773785,
    -0.00046446052148687905, 0.00034939361165105417, -0.0002633536495551932,
    0.00019885898700602698, -0.0001504063999160173, 0.00011393178617259052,
    -8.642320754869491e-05, 6.564143485541695e-05, -4.991697831321222e-05,
    3.8001927162546077e-05, -2.8961314711295418e-05, 2.209314682322636e-05,
    -1.686932038817502e-05, 1.2891834155415738e-05, -9.86023888809833e-06,
    7.54737769766621e-06, -5.781261162339443e-06, 4.431495660336892e-06,
    -3.399100216289112e-06, 2.6088513344058884e-06, -2.0035181213087346e-06,
    1.5395138373841213e-06, -1.1836108673737676e-06, 9.104555226369233e-07,
    -7.006854327413115e-07, 5.395016369359441e-07, -4.1558428389927703e-07,
    3.202683473607116e-07, -2.469163527350026e-07, 1.9044056002308284e-07,
    -1.469386541959237e-07, 1.1341573524768808e-07, -8.757198758072422e-08,
    6.764038400573971e-08, -5.2262540395907754e-08, 4.039368538745272e-08,
    -3.122986684565119e-08, 2.4152156136794418e-08, -1.868385388963757e-08,
    1.4457648827642462e-08, -1.1190400014929511e-08, 8.663762585260409e-09,
]

_NC_CACHE = None
LAST_EXEC_TIME_NS = None
LAST_TRACE_PATH = None


def _features(x, y):
    """Host-side float64 trig features -> bf16 hi/lo split pairs.

    U [128, NX]: row 0 = b0, rows 1..63 = b_k cos(2pi k x),
                 rows 64..126 = b_k sin(2pi k x), row 127 = 0.
    V [128, NY]: same but with unit coefficients.
    Returns (uh, ul, vh, vl) with u ~ uh + ul exactly (bf16 pairs).
    """
    import ml_dtypes

    bf16 = ml_dtypes.bfloat16

    xf = np.asarray(x, np.float32).reshape(-1).astype(np.float64)
    yf = np.asarray(y, np.float32).reshape(-1).astype(np.float64)
    ks = np.arange(1, KH + 1, dtype=np.float64)[:, None]
    bk = np.array(_BK, np.float64)[:, None]

    ang_x = (2.0 * np.pi) * ks * xf[None, :]
    u = np.empty((128, xf.size), np.float32)
    u[0] = _B0
    u[1 : KH + 1] = bk * np.cos(ang_x)
    u[KH + 1 : 2 * KH + 1] = bk * np.sin(ang_x)
    u[127] = 0.0

    ang_y = (2.0 * np.pi) * ks * yf[None, :]
    v = np.empty((128, yf.size), np.float32)
    v[0] = 1.0
    v[1 : KH + 1] = np.cos(ang_y)
    v[KH + 1 : 2 * KH + 1] = np.sin(ang_y)
    v[127] = 0.0

    uh = u.astype(bf16)
    ul = (u - uh.astype(np.float32)).astype(bf16)
    vh = v.astype(bf16)
    vl = (v - vh.astype(np.float32)).astype(bf16)
    return uh, ul, vh, vl


def _build():
    """Build + compile the per-core Bass/Tile kernel (cached)."""
    global _NC_CACHE
    if _NC_CACHE is not None:
        return _NC_CACHE

    from concourse import bacc, mybir
    import concourse.tile as tile

    f32 = mybir.dt.float32
    bf16 = mybir.dt.bfloat16

    nc = bacc.Bacc(
        "TRN2", target_bir_lowering=False, debug=False, num_devices=N_CORES
    )
    uxh_d = nc.dram_tensor("uxh", [128, MX], bf16, kind="ExternalInput").ap()
    uxl_d = nc.dram_tensor("uxl", [128, MX], bf16, kind="ExternalInput").ap()
    vyh_d = nc.dram_tensor("vyh", [128, NY], bf16, kind="ExternalInput").ap()
    vyl_d = nc.dram_tensor("vyl", [128, NY], bf16, kind="ExternalInput").ap()
    out_d = nc.dram_tensor("out", [MX, NY], f32, kind="ExternalOutput").ap()

    n_mt = MX // 128   # 8 row blocks
    n_ng = NY // 2048  # 4 col groups of 2048

    with tile.TileContext(nc) as tc:
        with (
            tc.tile_pool(name="wpool", bufs=1) as wpool,
            tc.tile_pool(name="vpool", bufs=2 * n_ng) as vpool,
            tc.tile_pool(name="pspool", bufs=2, space="PSUM") as pspool,
            tc.tile_pool(name="opool", bufs=2) as opool,
        ):
            uxh_t = wpool.tile([128, MX], bf16, name="uxh_t", tag="uxh_t")
            uxl_t = wpool.tile([128, MX], bf16, name="uxl_t", tag="uxl_t")
            nc.sync.dma_start(uxh_t[:], uxh_d[:])
            nc.sync.dma_start(uxl_t[:], uxl_d[:])
            vhs, vls = [], []
            for ng in range(n_ng):
                sl = slice(ng * 2048, (ng + 1) * 2048)
                vh_t = vpool.tile([128, 2048], bf16, name=f"vh_{ng}", tag="vy")
                vl_t = vpool.tile([128, 2048], bf16, name=f"vl_{ng}", tag="vy")
                nc.sync.dma_start(vh_t[:], vyh_d[:, sl])
                nc.sync.dma_start(vl_t[:], vyl_d[:, sl])
                vhs.append(vh_t)
                vls.append(vl_t)

            for m in range(n_mt):
                msl = slice(m * 128, (m + 1) * 128)
                out_t = opool.tile([128, NY], f32, name=f"out_{m}", tag="out_t")
                for ng in range(n_ng):
                    ps = pspool.tile(
                        [128, 2048], f32, name=f"ps_{m}_{ng}", tag="ps"
                    )
                    for s in range(4):
                        ssl = slice(s * 512, (s + 1) * 512)
                        # u.T v ~ uh.T vh + uh.T vl + ul.T vh  (fp32 PSUM)
                        nc.tensor.matmul(
                            ps[:, ssl], uxh_t[:, msl], vhs[ng][:, ssl],
                            start=True, stop=False,
                        )
                        nc.tensor.matmul(
                            ps[:, ssl], uxh_t[:, msl], vls[ng][:, ssl],
                            start=False, stop=False,
                        )
                        nc.tensor.matmul(
                            ps[:, ssl], uxl_t[:, msl], vhs[ng][:, ssl],
                            start=False, stop=True,
                        )
                    nc.scalar.activation(
                        out_t[:, ng * 2048 : (ng + 1) * 2048],
                        ps[:],
                        mybir.ActivationFunctionType.Exp,
                    )
                nc.sync.dma_start(out_d[msl, :], out_t[:])

    nc.compile()
    _NC_CACHE = nc
    return nc


def kernel(x: np.ndarray, y: np.ndarray) -> np.ndarray:
    global LAST_EXEC_TIME_NS, LAST_TRACE_PATH
    from concourse import bass_utils

    uh, ul, vh, vl = _features(x, y)
    nc = _build()

    in_maps = [
        {
            "uxh": np.ascontiguousarray(uh[:, i * MX : (i + 1) * MX]),
            "uxl": np.ascontiguousarray(ul[:, i * MX : (i + 1) * MX]),
            "vyh": vh,
            "vyl": vl,
        }
        for i in range(N_CORES)
    ]
    trace = bool(os.environ.get("BESSEL_TRACE"))
    res = bass_utils.run_bass_kernel_spmd(
        nc, in_maps, core_ids=list(range(N_CORES)), trace=trace
    )
    LAST_EXEC_TIME_NS = res.exec_time_ns
    if res.instructions_and_trace is not None:
        LAST_TRACE_PATH = res.instructions_and_trace[1]
    return np.concatenate(
        [res.results[i]["out"] for i in range(N_CORES)], axis=0
    )
